# revision 5
# baseline (speedup 1.0000x reference)
"""Batched Householder reflection: s_new[b] = s[b] - 2*(v[b]@s[b])/(v[b]@v[b]) * v[b].

Full inputs v, s: [512, 512] f32. Sharded batch-parallel across 8 NeuronCores
(64 rows per core). Per core: rows on SBUF partitions, K=512 on the free axis.
v and s shards are stacked host-side into one [2, 64, 512] DRAM tensor.

v2 schedule (from perfetto analysis of v1 @16.6us):
- 2 semaphores instead of 5: each semaphore costs a MOVE slot on every
  engine in the walrus main-BB prologue plus a sem_clear in the tail.
  sv counts v-loads + the whole DVE chain + stores; ss counts s-loads + nsq.
- v loads first (SP + pool q0), s second (ACT + pool q1): ACT's Square only
  needs v, so nsq overlaps the s transfer tail. Row split 40/24 because
  GpSimd's SWDGE desc-gen starts ~750ns late (Q7 dispatch) and serializes
  its two dma_starts.
- -2 folded into the dot accumulation (op0 scalar). DVE stt has no divide,
  so 1/nsq stays a dedicated DVE reciprocal.
  dotm2 = rowsum(-2*v*s)   (DVE stt accum_out)
  nsq   = rowsum(v*v)      (ACT Square accum_out, parallel with dot)
  coef  = dotm2 * (1/nsq)  (DVE reciprocal + stt)
  out   = coef*v + s       (DVE stt, per-partition scalar broadcast)
"""

import numpy as np

B, K = 512, 512
N_CORES = 8
B_LOC = B // N_CORES  # 64 rows per core

_nc = None


def _build():
    import concourse.bass as bass
    from concourse import mybir

    nc = bass.Bass("TRN2", debug=False, num_devices=N_CORES, num_swdge_queues=2)
    f32 = mybir.dt.float32

    vs = nc.dram_tensor("vs", [2, B_LOC, K], f32, kind="ExternalInput").ap()
    out = nc.dram_tensor("out", [B_LOC, K], f32, kind="ExternalOutput").ap()

    vs_t = nc.alloc_sbuf_tensor("vs_t", [B_LOC, 2, K], f32).ap()
    o_t = nc.alloc_sbuf_tensor("o_t", [B_LOC, K], f32).ap()
    junk_vs = nc.alloc_sbuf_tensor("junk_vs", [B_LOC, K], f32).ap()
    junk_vv = nc.alloc_sbuf_tensor("junk_vv", [B_LOC, K], f32).ap()
    warm = nc.alloc_sbuf_tensor("warm", [B_LOC, 1], f32).ap()
    dotm2 = nc.alloc_sbuf_tensor("dotm2", [B_LOC, 1], f32).ap()
    nsq = nc.alloc_sbuf_tensor("nsq", [B_LOC, 1], f32).ap()
    coef = nc.alloc_sbuf_tensor("coef", [B_LOC, 1], f32).ap()
    rcp = nc.alloc_sbuf_tensor("rcp", [B_LOC, 1], f32).ap()

    sv = nc.alloc_semaphore("sv")
    ss = nc.alloc_semaphore("ss")

    mult = mybir.AluOpType.mult
    add = mybir.AluOpType.add
    divide = mybir.AluOpType.divide
    Square = mybir.ActivationFunctionType.Square

    sp, act, ve, pl = nc.sync, nc.scalar, nc.vector, nc.gpsimd
    v_t = vs_t[:, 0, :]
    s_t = vs_t[:, 1, :]
    zero64 = nc.const_aps.scalar_like(0.0, dotm2[:])

    # ---- loads: v first (SP + pool q0), s second (ACT + pool q1) ----
    SPLIT = 40  # SP/ACT rows; pool takes the rest (its desc-gen starts late)
    sp.dma_start(out=vs_t[:SPLIT, 0, :], in_=vs[0, :SPLIT, :]).then_inc(sv, 16)
    act.dma_start(out=vs_t[:SPLIT, 1, :], in_=vs[1, :SPLIT, :]).then_inc(ss, 16)
    pl.dma_start(out=vs_t[SPLIT:, 0, :], in_=vs[0, SPLIT:, :]).then_inc(sv, 16)
    pl.dma_start(out=vs_t[SPLIT:, 1, :], in_=vs[1, SPLIT:, :]).then_inc(ss, 16)

    # ACT: prewarm the Square table while the DMAs are in flight
    act.activation(out=warm[:], in_=zero64, func=Square)

    # nsq = rowsum(v*v) on ACT as soon as v lands; ss -> 33 once s also landed
    act.wait_ge(sv, 32)
    act.activation(out=junk_vv[:], in_=v_t, func=Square, accum_out=nsq[:]).then_inc(
        ss, 1
    )

    # DVE chain. dotm2 = rowsum(-2*v*s); sv tracks DVE write visibility too.
    ve.wait_ge(sv, 32)
    ve.wait_ge(ss, 32)
    ve.scalar_tensor_tensor(
        out=junk_vs[:],
        in0=v_t,
        scalar=-2.0,
        in1=s_t,
        op0=mult,
        op1=mult,
        accum_out=dotm2[:],
    ).then_inc(sv, 1)
    ve.wait_ge(ss, 33)
    ve.reciprocal(out=rcp[:], in_=nsq[:]).then_inc(sv, 1)
    ve.wait_ge(sv, 34)
    ve.scalar_tensor_tensor(
        out=coef[:], in0=dotm2[:], scalar=1.0, in1=rcp[:], op0=mult, op1=mult
    ).then_inc(sv, 1)
    ve.wait_ge(sv, 35)
    ve.scalar_tensor_tensor(
        out=o_t[:],
        in0=v_t,
        scalar=coef[:],
        in1=s_t,
        op0=mult,
        op1=add,
    ).then_inc(sv, 1)

    # ---- stores: three streams (SP / ACT / Pool q0) ----
    # ACT's store issue is measurably slower (activation-pipe drain before
    # descriptor gen), so it gets the smallest chunk.
    sp.wait_ge(sv, 36)
    sp.dma_start(out=out[0:22, :], in_=o_t[0:22, :]).then_inc(sv, 16)
    act.wait_ge(sv, 36)
    act.dma_start(out=out[22:40, :], in_=o_t[22:40, :]).then_inc(sv, 16)
    pl.wait_ge(sv, 36)
    pl.dma_start(out=out[40:64, :], in_=o_t[40:64, :]).then_inc(sv, 16)

    # SP resets semaphores for re-execution (PJRT reuses the loaded NEFF).
    # sv=83 proves every waiter (ACT/Pool store waits included) has passed.
    sp.wait_ge(sv, 84)
    sp.sem_clear(sv)
    sp.sem_clear(ss)

    return nc


def kernel(i=None, v=None, s=None, **_):
    global _nc
    from concourse.bass_utils import run_bass_kernel_spmd

    if _nc is None:
        _nc = _build()

    v = np.asarray(v, dtype=np.float32)
    s = np.asarray(s, dtype=np.float32)
    in_maps = [
        {
            "vs": np.ascontiguousarray(
                np.stack(
                    [v[c * B_LOC : (c + 1) * B_LOC], s[c * B_LOC : (c + 1) * B_LOC]]
                )
            )
        }
        for c in range(N_CORES)
    ]
    res = run_bass_kernel_spmd(_nc, in_maps, core_ids=list(range(N_CORES)))
    return np.concatenate([r["out"] for r in res.results], axis=0)
